# revision 2
# baseline (speedup 1.0000x reference)
"""fp8 mixed-precision version of the fused band-attention block.

Same sharding/structure as kernel.py, but all 7 projection GEMMs run as
fp8e4 DoubleRow matmuls (2 k-chunks per instruction, 0.5 cyc/row): weights
are pre-scaled by 2^10 and cast to e4m3 host-side; activations are scaled
into e4m3 on the fly (x and t2 by 2^5, attention out by 2^5, ffn hidden by
2^3); every dequant folds into the existing PSUM-eviction multiply.
Attention internals (scores/softmax/AV) stay bf16.
"""

from contextlib import ExitStack, nullcontext

import numpy as np
import ml_dtypes

import concourse.bacc as bacc
import concourse.bass as bass
import concourse.mybir as mybir
import concourse.tile as tile
from concourse.bass_utils import run_bass_kernel_spmd
from concourse.masks import make_identity

BF = ml_dtypes.bfloat16
F8 = ml_dtypes.float8_e4m3
F32 = mybir.dt.float32
BF16 = mybir.dt.bfloat16
E4 = mybir.dt.float8e4
DR = mybir.MatmulPerfMode.DoubleRow

B, L, DIM, H, W, DFF = 2, 4096, 2048, 16, 128, 8192
HD = DIM // H          # 128
P = 128
NCORES = 8
OWN = (B * L) // NCORES  # 1024 tokens per core
EXT = OWN + W            # 1152 with halo
KC = DIM // P            # 16 k-chunks over model dim
KF = DFF // P            # 64 k-chunks over ffn dim
NBLK = OWN // W          # 8 query blocks per core
MC = EXT // P            # 9 token tiles
EPS = 1e-6
SCALE = float(HD) ** -0.5

SW = 1024.0   # weight fp8 scale
SX = 32.0     # x / t2 / attn-out fp8 scale
SH = 8.0      # ffn hidden fp8 scale
MQ = 1.0 / (SX * SW)          # dequant for x@W and t2@W psum
MO = 1.0 / (SX * SW)          # dequant for aoT@Wo psum
MH = 1.0 / (SH * SW)          # dequant for h@Wout psum

_CACHE = {}


def _build(n_loop=1, phases="123456", debug_dump=False):
    nc = bacc.Bacc("TRN2", target_bir_lowering=False, debug=False)

    xT = nc.dram_tensor("xT", [DIM, EXT], F32, kind="ExternalInput")
    halo_kT = nc.dram_tensor("halo_kT", [P, H, W], BF16, kind="ExternalInput")
    halo_v = nc.dram_tensor("halo_v", [W, DIM], BF16, kind="ExternalInput")
    wq_tl = nc.dram_tensor("wq_tl", [KC, P, KC, P], E4, kind="ExternalInput")
    wk_tl = nc.dram_tensor("wk_tl", [KC, P, KC, P], E4, kind="ExternalInput")
    wv_tl = nc.dram_tensor("wv_tl", [4, P, KC, 512], E4, kind="ExternalInput")
    wo_tl = nc.dram_tensor("wo_tl", [KC, P, KC, P], E4, kind="ExternalInput")
    wfa_tl = nc.dram_tensor("wfa_tl", [KF, P, KC, P], BF16, kind="ExternalInput")
    wfc_tl = nc.dram_tensor("wfc_tl", [KF, P, KC, P], BF16, kind="ExternalInput")
    wfo_tl = nc.dram_tensor("wfo_tl", [KC, P, KF, P], BF16, kind="ExternalInput")
    yT = nc.dram_tensor("yT", [DIM, OWN], F32, kind="ExternalOutput")
    dbg = {}
    if debug_dump:
        dbg["xq"] = nc.dram_tensor("dbg_xq", [P, KC, EXT], E4,
                                   kind="ExternalOutput")
        dbg["rstd1"] = nc.dram_tensor("dbg_rstd1", [1, EXT], F32,
                                      kind="ExternalOutput")
        if "2" in phases:
            dbg["vv"] = nc.dram_tensor("dbg_vv", [P, MC, DIM], BF16,
                                       kind="ExternalOutput")
        if "3" in phases:
            dbg["kT"] = nc.dram_tensor("dbg_kT", [P, H, EXT], BF16,
                                       kind="ExternalOutput")
        if "4" in phases:
            dbg["aoT"] = nc.dram_tensor("dbg_aoT", [P, KC, OWN], E4,
                                        kind="ExternalOutput")
        if "5" in phases:
            dbg["x2T"] = nc.dram_tensor("dbg_x2T", [DIM, OWN], F32,
                                        kind="ExternalOutput")
            dbg["rstd2"] = nc.dram_tensor("dbg_rstd2", [1, OWN], F32,
                                          kind="ExternalOutput")

    ext_cuts = [(0, 512), (512, 1024), (1024, EXT)]
    own_cuts = [(0, 512), (512, 1024)]
    wdma_start = nc.scalar.dma_start

    with tile.TileContext(nc) as tc, ExitStack() as top:
        dram = top.enter_context(tc.tile_pool(name="dram", bufs=1, space="DRAM"))
        x2T_d = dram.tile([DIM, OWN], F32, tag="x2T_d")
        x2b_d = dram.tile([DIM, OWN], BF16, tag="x2b_d")
        rstd1_d = dram.tile([1, EXT], F32, tag="rstd1_d")

        const = top.enter_context(tc.tile_pool(name="const", bufs=1))

        # band mask, additive: valid iff 1 <= j - p <= 128 (query p, window key j)
        mask = const.tile([P, 2 * W], F32)
        nc.gpsimd.memset(mask[:], 0.0)
        nc.gpsimd.affine_select(
            out=mask[:], in_=mask[:], compare_op=mybir.AluOpType.is_ge,
            fill=-1e4, base=-1, channel_multiplier=-1, pattern=[[1, 2 * W]])
        nc.gpsimd.affine_select(
            out=mask[:], in_=mask[:], compare_op=mybir.AluOpType.is_ge,
            fill=-1e4, base=W, channel_multiplier=1, pattern=[[-1, 2 * W]])

        ident = const.tile([P, P], BF16)
        make_identity(nc, ident[:])
        ones1 = const.tile([P, 1], BF16)
        nc.vector.memset(ones1[:], 1.0)
        # rstd1 here is m1 = rstd/(SX*SW): 1/m1 = sqrt(c1*ss + c1*DIM*eps)
        c1 = (SX * SW) ** 2 / DIM
        eps1_t = const.tile([1, 1], F32)
        nc.vector.memset(eps1_t[:], (SX * SW) ** 2 * EPS)
        # rstd2 is the plain 1/sqrt(ms+eps) (bf16 FFN, no fp8 scaling)
        c2 = 1.0 / DIM
        eps2_t = const.tile([1, 1], F32)
        nc.vector.memset(eps2_t[:], EPS)

        rstd2_pool = top.enter_context(tc.tile_pool(name="rstd2p", bufs=1))
        rstd2_b = rstd2_pool.tile([P, OWN], F32, tag="rstd2_b")

        with (tc.For_i(0, n_loop, 1) if n_loop > 1 else nullcontext()):
            with ExitStack() as mha:  # Ph1..Ph5 buffers
                kv_pool = mha.enter_context(tc.tile_pool(name="kv", bufs=1))
                kT = kv_pool.tile([P, H, EXT], BF16, tag="kT")
                vv = kv_pool.tile([P, MC, DIM], BF16, tag="vv")
                xq_pool = mha.enter_context(tc.tile_pool(name="xqp", bufs=1))
                xq = xq_pool.tile([P, KC, EXT], E4, tag="xq")
                rs_pool = mha.enter_context(tc.tile_pool(name="rsp", bufs=1))
                rstd1_b = rs_pool.tile([P, EXT], F32, tag="rstd1_b")
                rstd1_c = rs_pool.tile([P, MC], F32, tag="rstd1_c")

                # ---- Phase 1: stream x, cast to fp8 (x*SX), rmsnorm1 stats ---
                if "1" not in phases:
                    raise ValueError("phase 1 is required")
                with ExitStack() as ph:
                    ph.enter_context(nc.named_scope("ph1_xnorm"))
                    xf_pool = ph.enter_context(tc.tile_pool(name="xf", bufs=3))
                    xsq_pool = ph.enter_context(tc.tile_pool(name="xsq", bufs=3))
                    ss_ps = ph.enter_context(
                        tc.tile_pool(name="ss_ps", bufs=1, space="PSUM"))
                    ss = [ss_ps.tile([1, c1_ - c0_], F32, tag=f"ss{i}",
                                     name=f"ss{i}")
                          for i, (c0_, c1_) in enumerate(ext_cuts)]
                    for kc in range(KC):
                        xt = xf_pool.tile([P, EXT], F32, tag="xf", name="xf")
                        xeng = nc.sync if kc % 2 == 0 else nc.scalar
                        xeng.dma_start(out=xt[:],
                                       in_=xT[kc * P:(kc + 1) * P, :])
                        nc.scalar.activation(
                            out=xq[:, kc, :], in_=xt[:],
                            func=mybir.ActivationFunctionType.Copy, scale=SX)
                        xsq = xsq_pool.tile([P, EXT], BF16, tag="xsq",
                                            name="xsq")
                        nc.vector.tensor_mul(xsq[:], xt[:], xt[:])
                        for i, (c0_, c1_) in enumerate(ext_cuts):
                            nc.tensor.matmul(ss[i][:], ones1[:], xsq[:, c0_:c1_],
                                             start=(kc == 0),
                                             stop=(kc == KC - 1),
                                             skip_group_check=True)
                    rstd1 = rs_pool.tile([1, EXT], F32, tag="rstd1")
                    for i, (c0_, c1_) in enumerate(ext_cuts):
                        nc.scalar.activation(
                            out=rstd1[:, c0_:c1_], in_=ss[i][:],
                            func=mybir.ActivationFunctionType.Sqrt,
                            bias=eps1_t[:], scale=c1)
                    nc.vector.reciprocal(rstd1[:], rstd1[:])
                    nc.gpsimd.partition_broadcast(rstd1_b[:], rstd1[:])
                    # rstd as a (tok%128, tile) column view, via DRAM roundtrip
                    nc.sync.dma_start(out=rstd1_d[:], in_=rstd1[:])
                    nc.sync.dma_start(
                        out=rstd1_c[:],
                        in_=rstd1_d.rearrange("o (m p) -> (o p) m", p=P))
                    if debug_dump:
                        nc.sync.dma_start(out=dbg["xq"][:], in_=xq[:])
                        nc.sync.dma_start(out=dbg["rstd1"][:], in_=rstd1[:])

                # ---- Phase 2: V = (x.T @ Wv) * m1  (tok x dim layout) --------
                if "2" in phases:
                    with ExitStack() as ph:
                        ph.enter_context(nc.named_scope("ph2_v"))
                        wv_pool = ph.enter_context(tc.tile_pool(name="wv", bufs=2))
                        ps_pool = ph.enter_context(
                            tc.tile_pool(name="v_ps", bufs=4, space="PSUM"))
                        for ncol in range(4):
                            wv_t = wv_pool.tile([P, KC, 512], E4, tag="wv",
                                                name="wv")
                            wdma_start(out=wv_t[:], in_=wv_tl[ncol])
                            for mc in range(MC):
                                ps = ps_pool.tile([P, 512], F32, tag="vps",
                                                  name="vps")
                                for j in range(KC // 2):
                                    nc.tensor.matmul(
                                        ps[:],
                                        xq[:, 2 * j:2 * j + 2,
                                           mc * P:(mc + 1) * P],
                                        wv_t[:, 2 * j:2 * j + 2, :],
                                        start=(j == 0), stop=(j == KC // 2 - 1),
                                        perf_mode=DR)
                                nc.vector.tensor_scalar_mul(
                                    vv[:, mc, ncol * 512:(ncol + 1) * 512], ps[:],
                                    rstd1_c[:, mc:mc + 1])
                        hv = wv_pool.tile([P, DIM], BF16, tag="halo_v")
                        nc.sync.dma_start(out=hv[:], in_=halo_v[:])
                        nc.vector.tensor_add(vv[:, 0, :], vv[:, 0, :], hv[:])
                        if debug_dump:
                            nc.sync.dma_start(out=dbg["vv"][:], in_=vv[:])

                # ---- Phase 3: K = (Wk.T @ x) * m1 (dim x tok layout) ---------
                if "3" in phases:
                    with ExitStack() as ph:
                        ph.enter_context(nc.named_scope("ph3_k"))
                        w_pool = ph.enter_context(tc.tile_pool(name="wkp", bufs=3))
                        ps_pool = ph.enter_context(
                            tc.tile_pool(name="k_ps", bufs=4, space="PSUM"))
                        for oc in range(KC):
                            wk_t = w_pool.tile([P, KC, P], E4, tag="wk",
                                               name="wk")
                            wdma_start(out=wk_t[:], in_=wk_tl[oc])
                            for (c0, c1_) in ext_cuts:
                                ps = ps_pool.tile([P, 512], F32, tag="kps",
                                                  name="kps")
                                for j in range(KC // 2):
                                    nc.tensor.matmul(
                                        ps[:, :c1_ - c0],
                                        wk_t[:, 2 * j:2 * j + 2, :],
                                        xq[:, 2 * j:2 * j + 2, c0:c1_],
                                        start=(j == 0), stop=(j == KC // 2 - 1),
                                        perf_mode=DR)
                                nc.vector.tensor_mul(kT[:, oc, c0:c1_],
                                                     ps[:, :c1_ - c0],
                                                     rstd1_b[:, c0:c1_])
                        hk = w_pool.tile([P, H, W], BF16, tag="halo_k")
                        nc.sync.dma_start(out=hk[:], in_=halo_kT[:])
                        nc.vector.tensor_add(kT[:, :, 0:W], kT[:, :, 0:W], hk[:])
                        if debug_dump:
                            nc.sync.dma_start(out=dbg["kT"][:], in_=kT[:])

                # ---- Phase 4: per head: Q then band attention ----------------
                if "4" in phases:
                    ao_pool = mha.enter_context(tc.tile_pool(name="aop", bufs=1))
                    aoT = ao_pool.tile([P, KC, OWN], E4, tag="aoT")
                    with ExitStack() as ph:
                        ph.enter_context(nc.named_scope("ph4_attn"))
                        w_pool = ph.enter_context(tc.tile_pool(name="wqp", bufs=3))
                        qh_pool = ph.enter_context(tc.tile_pool(name="qhp", bufs=2))
                        sm_pool = ph.enter_context(tc.tile_pool(name="smp", bufs=3))
                        p_pool = ph.enter_context(tc.tile_pool(name="ppp", bufs=3))
                        q_ps = ph.enter_context(
                            tc.tile_pool(name="q_ps", bufs=2, space="PSUM"))
                        sc_ps = ph.enter_context(
                            tc.tile_pool(name="sc_ps", bufs=2, space="PSUM"))
                        tp_ps = ph.enter_context(
                            tc.tile_pool(name="tp_ps", bufs=2, space="PSUM"))
                        av_ps = ph.enter_context(
                            tc.tile_pool(name="av_ps", bufs=2, space="PSUM"))
                        for h in range(H):
                            wq_t = w_pool.tile([P, KC, P], E4, tag="wq",
                                               name="wq")
                            wdma_start(out=wq_t[:], in_=wq_tl[h])
                            qh = qh_pool.tile([P, OWN], BF16, tag="qh", name="qh")
                            for (c0, c1_) in own_cuts:
                                ps = q_ps.tile([P, 512], F32, tag="qps", name="qps")
                                for j in range(KC // 2):
                                    nc.tensor.matmul(
                                        ps[:],
                                        wq_t[:, 2 * j:2 * j + 2, :],
                                        xq[:, 2 * j:2 * j + 2, W + c0:W + c1_],
                                        start=(j == 0), stop=(j == KC // 2 - 1),
                                        perf_mode=DR)
                                nc.vector.tensor_mul(qh[:, c0:c1_], ps[:],
                                                     rstd1_b[:, W + c0:W + c1_])
                            for n in range(NBLK):
                                sc = sc_ps.tile([P, 2 * W], F32, tag="sc",
                                                name="sc")
                                nc.tensor.matmul(sc[:], qh[:, n * W:(n + 1) * W],
                                                 kT[:, h, n * W:n * W + 2 * W],
                                                 start=True, stop=True)
                                sm = sm_pool.tile([P, 2 * W], F32, tag="sm",
                                                  name="sm")
                                nc.vector.scalar_tensor_tensor(
                                    out=sm[:], in0=sc[:], scalar=SCALE, in1=mask[:],
                                    op0=mybir.AluOpType.mult,
                                    op1=mybir.AluOpType.add)
                                pe = p_pool.tile([P, 2 * W], BF16, tag="pe",
                                                 name="pe")
                                sumexp = sm_pool.tile([P, 1], F32, tag="sumexp",
                                                      name="sumexp")
                                nc.scalar.activation(
                                    out=pe[:], in_=sm[:],
                                    func=mybir.ActivationFunctionType.Exp,
                                    accum_out=sumexp[:])
                                recip = sm_pool.tile([P, 1], F32, tag="recip",
                                                     name="recip")
                                nc.vector.reciprocal(recip[:], sumexp[:])
                                pn = p_pool.tile([P, 2 * W], BF16, tag="pn",
                                                 name="pn")
                                nc.vector.tensor_scalar_mul(pn[:], pe[:], recip[:])
                                pT = p_pool.tile([P, 2, W], BF16, tag="pT",
                                                 name="pT")
                                for c in range(2):
                                    tp = tp_ps.tile([P, W], BF16, tag="tp",
                                                    name="tp")
                                    nc.tensor.transpose(
                                        tp[:], pn[:, c * W:(c + 1) * W], ident[:])
                                    nc.vector.tensor_copy(pT[:, c, :], tp[:])
                                av = av_ps.tile([P, W], F32, tag="av", name="av")
                                for c in range(2):
                                    nc.tensor.matmul(
                                        av[:], vv[:, n + c, h * HD:(h + 1) * HD],
                                        pT[:, c, :], start=(c == 0), stop=(c == 1))
                                nc.scalar.activation(
                                    out=aoT[:, h, n * W:(n + 1) * W], in_=av[:],
                                    func=mybir.ActivationFunctionType.Copy,
                                    scale=SX)

                if debug_dump and "4" in phases:
                    nc.sync.dma_start(out=dbg["aoT"][:], in_=aoT[:])
                # ---- Phase 5: O proj + residual -> x2 (DRAM f32+bf16), ------
                if "5" in phases:
                    # ---- fused rmsnorm2 stats -----------------------------------
                    with ExitStack() as ph:
                        ph.enter_context(nc.named_scope("ph5_o"))
                        w_pool = ph.enter_context(tc.tile_pool(name="wop", bufs=3))
                        st_pool = ph.enter_context(tc.tile_pool(name="ost", bufs=3))
                        ps_pool = ph.enter_context(
                            tc.tile_pool(name="o_ps", bufs=3, space="PSUM"))
                        ss_ps = ph.enter_context(
                            tc.tile_pool(name="ss2_ps", bufs=1, space="PSUM"))
                        ss2 = [ss_ps.tile([1, 512], F32, tag=f"ss2_{i}",
                                          name=f"ss2_{i}") for i in range(2)]
                        for oc in range(KC):
                            wo_t = w_pool.tile([P, KC, P], E4, tag="wo", name="wo")
                            wdma_start(out=wo_t[:], in_=wo_tl[oc])
                            for i, (c0, c1_) in enumerate(own_cuts):
                                ps = ps_pool.tile([P, 512], F32, tag="ops", name="ops")
                                for j in range(KC // 2):
                                    nc.tensor.matmul(ps[:],
                                                     wo_t[:, 2 * j:2 * j + 2, :],
                                                     aoT[:, 2 * j:2 * j + 2, c0:c1_],
                                                     start=(j == 0),
                                                     stop=(j == KC // 2 - 1),
                                                     perf_mode=DR)
                                xres = st_pool.tile([P, 512], F32, tag="xres",
                                                    name="xres")
                                nc.sync.dma_start(
                                    out=xres[:],
                                    in_=xT[oc * P:(oc + 1) * P, W + c0:W + c1_])
                                x2 = st_pool.tile([P, 512], F32, tag="x2", name="x2")
                                nc.vector.scalar_tensor_tensor(
                                    out=x2[:], in0=ps[:], scalar=MO, in1=xres[:],
                                    op0=mybir.AluOpType.mult,
                                    op1=mybir.AluOpType.add)
                                nc.sync.dma_start(
                                    out=x2T_d[oc * P:(oc + 1) * P, c0:c1_], in_=x2[:])
                                xb2s = st_pool.tile([P, 512], BF16, tag="xb2s",
                                                    name="xb2s")
                                nc.scalar.copy(xb2s[:], x2[:])
                                nc.sync.dma_start(
                                    out=x2b_d[oc * P:(oc + 1) * P, c0:c1_],
                                    in_=xb2s[:])
                                xsq = st_pool.tile([P, 512], BF16, tag="xsq2",
                                                   name="xsq2")
                                nc.vector.tensor_mul(xsq[:], x2[:], x2[:])
                                nc.tensor.matmul(ss2[i][:], ones1[:], xsq[:],
                                                 start=(oc == 0), stop=(oc == KC - 1),
                                                 skip_group_check=True)
                        rstd2 = st_pool.tile([1, OWN], F32, tag="rstd2")
                        for i, (c0, c1_) in enumerate(own_cuts):
                            nc.scalar.activation(out=rstd2[:, c0:c1_], in_=ss2[i][:],
                                                 func=mybir.ActivationFunctionType.Sqrt,
                                                 bias=eps2_t[:], scale=c2)
                        nc.vector.reciprocal(rstd2[:], rstd2[:])
                        nc.gpsimd.partition_broadcast(rstd2_b[:], rstd2[:])
                        if debug_dump:
                            nc.sync.dma_start(out=dbg["rstd2"][:], in_=rstd2[:])
                            nc.sync.dma_start(out=dbg["x2T"][:], in_=x2T_d[:])

            # ---- Phase 6: SwiGLU FFN + residual ------------------------------
            if "6" in phases:
                # Single-pass bf16 FFN: weights stream once; hbuf for all
                # OWN tokens stays resident (128KB/partition); each weight
                # tile serves both 512-token halves (cut-inner loops).
                with ExitStack() as ph:
                    h_pool = ph.enter_context(tc.tile_pool(name="hbufp", bufs=1))
                    hbuf = h_pool.tile([P, KF, OWN], BF16, tag="hbuf")

                    with ExitStack() as phfc:
                        phfc.enter_context(nc.named_scope("ph6a_fc"))
                        t2_pool = phfc.enter_context(
                            tc.tile_pool(name="t2p", bufs=1))
                        xb_pool = phfc.enter_context(
                            tc.tile_pool(name="xbp6", bufs=3))
                        wf_pool = phfc.enter_context(
                            tc.tile_pool(name="wfp", bufs=3))
                        s_pool = phfc.enter_context(
                            tc.tile_pool(name="silp", bufs=3))
                        ps12_pool = phfc.enter_context(
                            tc.tile_pool(name="f_ps12", bufs=2, space="PSUM"))

                        t2 = t2_pool.tile([P, KC, OWN], BF16, tag="t2")
                        for kc in range(KC):
                            xbs = xb_pool.tile([P, OWN], BF16, tag="xbs",
                                               name="xbs")
                            nc.sync.dma_start(out=xbs[:],
                                              in_=x2b_d[kc * P:(kc + 1) * P, :])
                            nc.vector.tensor_mul(t2[:, kc, :], xbs[:],
                                                 rstd2_b[:])

                        for oc in range(KF):
                            wfa_t = wf_pool.tile([P, KC, P], BF16, tag="wfa",
                                                 name="wfa")
                            nc.scalar.dma_start(out=wfa_t[:], in_=wfa_tl[oc])
                            wfc_t = wf_pool.tile([P, KC, P], BF16, tag="wfc",
                                                 name="wfc")
                            nc.sync.dma_start(out=wfc_t[:], in_=wfc_tl[oc])
                            for (c0, c1_) in own_cuts:
                                ps1 = ps12_pool.tile([P, 512], F32, tag="ps1",
                                                     name="ps1")
                                for kc in range(KC):
                                    nc.tensor.matmul(ps1[:], wfa_t[:, kc, :],
                                                     t2[:, kc, c0:c1_],
                                                     start=(kc == 0),
                                                     stop=(kc == KC - 1))
                                sil = s_pool.tile([P, 512], BF16, tag="sil",
                                                  name="sil")
                                nc.scalar.activation(
                                    out=sil[:], in_=ps1[:],
                                    func=mybir.ActivationFunctionType.Silu)
                                ps2 = ps12_pool.tile([P, 512], F32, tag="ps2",
                                                     name="ps2")
                                for kc in range(KC):
                                    nc.tensor.matmul(ps2[:], wfc_t[:, kc, :],
                                                     t2[:, kc, c0:c1_],
                                                     start=(kc == 0),
                                                     stop=(kc == KC - 1))
                                nc.vector.tensor_mul(hbuf[:, oc, c0:c1_],
                                                     ps2[:], sil[:])

                    with ExitStack() as phfo:
                        phfo.enter_context(nc.named_scope("ph6b_out"))
                        wfo_pool = phfo.enter_context(
                            tc.tile_pool(name="wfop", bufs=2))
                        y_pool = phfo.enter_context(
                            tc.tile_pool(name="yp", bufs=3))
                        ps3_pool = phfo.enter_context(
                            tc.tile_pool(name="f_ps3", bufs=3, space="PSUM"))
                        for oc in range(KC):
                            wfo_t = wfo_pool.tile([P, KF, P], BF16, tag="wfo",
                                                  name="wfo")
                            weng = nc.scalar if oc % 2 == 0 else nc.sync
                            weng.dma_start(out=wfo_t[:], in_=wfo_tl[oc])
                            for (c0, c1_) in own_cuts:
                                ps = ps3_pool.tile([P, 512], F32, tag="ps3",
                                                   name="ps3")
                                for kc in range(KF):
                                    nc.tensor.matmul(ps[:], wfo_t[:, kc, :],
                                                     hbuf[:, kc, c0:c1_],
                                                     start=(kc == 0),
                                                     stop=(kc == KF - 1))
                                x2s = y_pool.tile([P, 512], F32, tag="x2res",
                                                  name="x2res")
                                nc.scalar.dma_start(
                                    out=x2s[:],
                                    in_=x2T_d[oc * P:(oc + 1) * P, c0:c1_])
                                yt = y_pool.tile([P, 512], F32, tag="yt",
                                                 name="yt")
                                nc.vector.tensor_add(yt[:], ps[:], x2s[:])
                                nc.gpsimd.dma_start(
                                    out=yT[oc * P:(oc + 1) * P, c0:c1_],
                                    in_=yt[:])

    nc.compile()
    return nc


def _tile_w(wt, nkc, noc, ocw):
    """(din, dout) -> (dout//ocw, 128, din//128, ocw) so each [oc] is contiguous."""
    return np.ascontiguousarray(
        wt.reshape(nkc, P, noc, ocw).transpose(2, 1, 0, 3))


def _q8(wt):
    return np.clip(wt * SW, -240.0, 240.0).astype(F8)


def _q8_hilo(wt):
    s = wt * SW
    hi = np.clip(s, -240.0, 240.0).astype(F8)
    lo = (s - hi.astype(np.float32)).astype(F8)
    return hi, lo


def _group4(tl):
    """[KF, P, KC, P] -> [KF//4, P, 4, KC, P]: one DMA covers 4 ocs
    (8KB per-partition lines instead of 2KB)."""
    return np.ascontiguousarray(
        tl.reshape(KF // 4, 4, P, KC, P).transpose(0, 2, 1, 3, 4))


def _prep_inputs(x, wq, wk, wv, wo, last_k_init, last_v_init,
                 w_fc, w_fc_act, w_fc_out, g_mha, g_ffn):
    wq_t = _q8((wq * g_mha[None, :]).T)
    wk_t = _q8((wk * g_mha[None, :]).T)
    wv_t = _q8((wv * g_mha[None, :]).T)
    wo_t = _q8(wo.T)
    wfa_t = ((w_fc_act * g_ffn[None, :]).T).astype(BF)
    wfc_t = ((w_fc * g_ffn[None, :]).T).astype(BF)
    wfo_t = w_fc_out.T.astype(BF)

    shared = {
        "wq_tl": _tile_w(wq_t, KC, KC, P),
        "wk_tl": _tile_w(wk_t, KC, KC, P),
        "wv_tl": _tile_w(wv_t, KC, 4, 512),
        "wo_tl": _tile_w(wo_t, KC, KC, P),
        "wfa_tl": _tile_w(wfa_t, KC, KF, P),
        "wfc_tl": _tile_w(wfc_t, KC, KF, P),
        "wfo_tl": _tile_w(wfo_t, KF, KC, P),
    }

    # halo k/v for first-chunk cores, from last_k/v_init
    hk = np.zeros((W, H, HD), np.float32)
    hk[1:W] = last_k_init
    halo_kT0 = np.ascontiguousarray(hk.transpose(2, 1, 0)).astype(BF)  # (hd,h,j)
    hv = np.zeros((W, DIM), np.float32)
    hv[1:W] = last_v_init.reshape(W - 1, DIM)
    halo_v0 = hv.astype(BF)
    halo_kTz = np.zeros_like(halo_kT0)
    halo_vz = np.zeros_like(halo_v0)

    in_maps = []
    for c in range(NCORES):
        b, s = divmod(c * OWN, L)
        xe = np.zeros((EXT, DIM), np.float32)
        xe[W:] = x[b, s:s + OWN]
        if s > 0:
            xe[:W] = x[b, s - W:s]
        m = dict(shared)
        m["xT"] = np.ascontiguousarray(xe.T)
        m["halo_kT"] = halo_kT0 if s == 0 else halo_kTz
        m["halo_v"] = halo_v0 if s == 0 else halo_vz
        in_maps.append(m)
    return in_maps


def _run(inputs, trace=False):
    if "nc" not in _CACHE:
        _CACHE["nc"] = _build()
    nc = _CACHE["nc"]
    in_maps = _prep_inputs(**{k: np.asarray(v) for k, v in inputs.items()})
    res = run_bass_kernel_spmd(nc, in_maps, core_ids=list(range(NCORES)),
                               trace=trace)
    y = np.empty((B, L, DIM), np.float32)
    for c in range(NCORES):
        b, s = divmod(c * OWN, L)
        y[b, s:s + OWN] = res.results[c]["yT"].T
    return y, res


def kernel(**inputs):
    y, _ = _run(inputs, trace=False)
    return y



# revision 8
# speedup vs baseline: 1.0178x; 1.0178x over previous
"""Fused band-attention block (fp8 QKVO + bf16 FFN), 8-core data-parallel.

v2 changes over the original baseline:
- xT input streamed as bf16 (halves x DMA; residual read in bf16)
- x2 (post-attention residual) kept in SBUF as bf16 — the f32/bf16 DRAM
  roundtrip between phase 5 and phase 6 is gone.  Phase 6a normalizes it
  in place (t2 = x2*rstd2); phase 6b recovers the residual as t2*(1/rstd2).
- wfo (FFN down-proj) weight tiles stream as two half-tiles whose SBUF
  space is reserved during phase 6a, so the first tiles prefetch during
  the fc/gate GEMMs instead of stalling the 6a->6b transition.
- optional interleaved psum-bank matmul ordering (ffn_ilv/qkv_ilv).
"""

from contextlib import ExitStack, nullcontext

import numpy as np
import ml_dtypes

import concourse.bacc as bacc
import concourse.bass as bass
import concourse.mybir as mybir
import concourse.tile as tile
from concourse.bass_utils import run_bass_kernel_spmd
from concourse.masks import make_identity

BF = ml_dtypes.bfloat16
F8 = ml_dtypes.float8_e4m3
F32 = mybir.dt.float32
BF16 = mybir.dt.bfloat16
E4 = mybir.dt.float8e4
DR = mybir.MatmulPerfMode.DoubleRow

B, L, DIM, H, W, DFF = 2, 4096, 2048, 16, 128, 8192
HD = DIM // H          # 128
P = 128
NCORES = 8
OWN = (B * L) // NCORES  # 1024 tokens per core
EXT = OWN + W            # 1152 with halo
KC = DIM // P            # 16 k-chunks over model dim
KF = DFF // P            # 64 k-chunks over ffn dim
NBLK = OWN // W          # 8 query blocks per core
MC = EXT // P            # 9 token tiles
EPS = 1e-6
SCALE = float(HD) ** -0.5

SW = 1024.0   # weight fp8 scale
SX = 32.0     # x / attn-out fp8 scale
MQ = 1.0 / (SX * SW)          # dequant for x@W psum
MO = 1.0 / (SX * SW)          # dequant for aoT@Wo psum

_CACHE = {}


def _build(n_loop=1, phases="123456", ffn_ilv=False, qkv_ilv=False):
    nc = bacc.Bacc("TRN2", target_bir_lowering=False, debug=False)

    xT = nc.dram_tensor("xT", [DIM, EXT], BF16, kind="ExternalInput")
    halo_kT = nc.dram_tensor("halo_kT", [P, H, W], BF16, kind="ExternalInput")
    halo_v = nc.dram_tensor("halo_v", [W, DIM], BF16, kind="ExternalInput")
    wq_tl = nc.dram_tensor("wq_tl", [KC, P, KC, P], E4, kind="ExternalInput")
    wk_tl = nc.dram_tensor("wk_tl", [KC, P, KC, P], E4, kind="ExternalInput")
    wv_tl = nc.dram_tensor("wv_tl", [4, P, KC, 512], E4, kind="ExternalInput")
    wo_tl = nc.dram_tensor("wo_tl", [KC, P, KC, P], E4, kind="ExternalInput")
    wfa_tl = nc.dram_tensor("wfa_tl", [KF, P, KC, P], BF16, kind="ExternalInput")
    wfc_tl = nc.dram_tensor("wfc_tl", [KF, P, KC, P], BF16, kind="ExternalInput")
    wfo_tl = nc.dram_tensor("wfo_tl", [KC, 4, P, KF // 4, P], BF16,
                            kind="ExternalInput")
    yT = nc.dram_tensor("yT", [DIM, OWN], F32, kind="ExternalOutput")

    ext_cuts = [(0, 512), (512, 1024), (1024, EXT)]
    own_cuts = [(0, 512), (512, 1024)]
    wdma_start = nc.scalar.dma_start

    with tile.TileContext(nc) as tc, ExitStack() as top:
        dram = top.enter_context(tc.tile_pool(name="dram", bufs=1, space="DRAM"))
        rstd1_d = dram.tile([1, EXT], F32, tag="rstd1_d")

        const = top.enter_context(tc.tile_pool(name="const", bufs=1))

        # band mask, additive: valid iff 1 <= j - p <= 128 (query p, window key j)
        mask = const.tile([P, 2 * W], F32)
        nc.gpsimd.memset(mask[:], 0.0)
        nc.gpsimd.affine_select(
            out=mask[:], in_=mask[:], compare_op=mybir.AluOpType.is_ge,
            fill=-1e4, base=-1, channel_multiplier=-1, pattern=[[1, 2 * W]])
        nc.gpsimd.affine_select(
            out=mask[:], in_=mask[:], compare_op=mybir.AluOpType.is_ge,
            fill=-1e4, base=W, channel_multiplier=1, pattern=[[-1, 2 * W]])

        ident = const.tile([P, P], BF16)
        make_identity(nc, ident[:])
        ones1 = const.tile([P, 1], BF16)
        nc.vector.memset(ones1[:], 1.0)
        # rstd1 here is m1 = rstd/(SX*SW): 1/m1 = sqrt(c1*ss + c1*DIM*eps)
        c1 = (SX * SW) ** 2 / DIM
        eps1_t = const.tile([1, 1], F32)
        nc.vector.memset(eps1_t[:], (SX * SW) ** 2 * EPS)
        # rstd2 is the plain 1/sqrt(ms+eps) (bf16 FFN, no fp8 scaling)
        c2 = 1.0 / DIM
        eps2_t = const.tile([1, 1], F32)
        nc.vector.memset(eps2_t[:], EPS)

        rstd2_pool = top.enter_context(tc.tile_pool(name="rstd2p", bufs=1))
        rstd2_b = rstd2_pool.tile([P, OWN], F32, tag="rstd2_b")
        x2_pool = top.enter_context(tc.tile_pool(name="x2p", bufs=1))
        x2sb = x2_pool.tile([P, KC, OWN], BF16, tag="x2sb")

        with (tc.For_i(0, n_loop, 1) if n_loop > 1 else nullcontext()):
            with ExitStack() as mha:  # Ph1..Ph5 buffers
                kv_pool = mha.enter_context(tc.tile_pool(name="kv", bufs=1))
                kT = kv_pool.tile([P, H, EXT], BF16, tag="kT")
                vv = kv_pool.tile([P, MC, DIM], BF16, tag="vv")
                xq_pool = mha.enter_context(tc.tile_pool(name="xqp", bufs=1))
                xq = xq_pool.tile([P, KC, EXT], E4, tag="xq")
                rs_pool = mha.enter_context(tc.tile_pool(name="rsp", bufs=1))
                rstd1_b = rs_pool.tile([P, EXT], F32, tag="rstd1_b")
                rstd1_c = rs_pool.tile([P, MC], F32, tag="rstd1_c")

                # ---- Phase 1: stream x (bf16), cast to fp8 (x*SX), rmsnorm1 --
                if "1" not in phases:
                    raise ValueError("phase 1 is required")
                with ExitStack() as ph:
                    ph.enter_context(nc.named_scope("ph1_xnorm"))
                    xf_pool = ph.enter_context(tc.tile_pool(name="xf", bufs=3))
                    xsq_pool = ph.enter_context(tc.tile_pool(name="xsq", bufs=3))
                    ss_ps = ph.enter_context(
                        tc.tile_pool(name="ss_ps", bufs=1, space="PSUM"))
                    ss = [ss_ps.tile([1, c1_ - c0_], F32, tag=f"ss{i}",
                                     name=f"ss{i}")
                          for i, (c0_, c1_) in enumerate(ext_cuts)]
                    for kc in range(KC):
                        xt = xf_pool.tile([P, EXT], BF16, tag="xf", name="xf")
                        xeng = nc.sync if kc % 2 == 0 else nc.scalar
                        xeng.dma_start(out=xt[:],
                                       in_=xT[kc * P:(kc + 1) * P, :])
                        nc.scalar.activation(
                            out=xq[:, kc, :], in_=xt[:],
                            func=mybir.ActivationFunctionType.Copy, scale=SX)
                        xsq = xsq_pool.tile([P, EXT], BF16, tag="xsq",
                                            name="xsq")
                        nc.vector.tensor_mul(xsq[:], xt[:], xt[:])
                        for i, (c0_, c1_) in enumerate(ext_cuts):
                            nc.tensor.matmul(ss[i][:], ones1[:], xsq[:, c0_:c1_],
                                             start=(kc == 0),
                                             stop=(kc == KC - 1),
                                             skip_group_check=True)
                    rstd1 = rs_pool.tile([1, EXT], F32, tag="rstd1")
                    for i, (c0_, c1_) in enumerate(ext_cuts):
                        nc.scalar.activation(
                            out=rstd1[:, c0_:c1_], in_=ss[i][:],
                            func=mybir.ActivationFunctionType.Sqrt,
                            bias=eps1_t[:], scale=c1)
                    nc.vector.reciprocal(rstd1[:], rstd1[:])
                    nc.gpsimd.partition_broadcast(rstd1_b[:], rstd1[:])
                    # rstd as a (tok%128, tile) column view, via DRAM roundtrip
                    nc.sync.dma_start(out=rstd1_d[:], in_=rstd1[:])
                    nc.sync.dma_start(
                        out=rstd1_c[:],
                        in_=rstd1_d.rearrange("o (m p) -> (o p) m", p=P))

                # ---- Phase 2: V = (x.T @ Wv) * m1  (tok x dim layout) --------
                if "2" in phases:
                    with ExitStack() as ph:
                        ph.enter_context(nc.named_scope("ph2_v"))
                        wv_pool = ph.enter_context(tc.tile_pool(name="wv", bufs=2))
                        ps_pool = ph.enter_context(
                            tc.tile_pool(name="v_ps", bufs=3 if qkv_ilv else 4,
                                         space="PSUM"))
                        for ncol in range(4):
                            wv_t = wv_pool.tile([P, KC, 512], E4, tag="wv",
                                                name="wv")
                            wdma_start(out=wv_t[:], in_=wv_tl[ncol])
                            if qkv_ilv:
                                for mc2 in range(0, MC + 1, 2):
                                    mcs = [m for m in (mc2, mc2 + 1) if m < MC]
                                    if not mcs:
                                        continue
                                    pss = {m: ps_pool.tile(
                                        [P, 512], F32, tag=f"vps{m % 2}",
                                        name=f"vps{m % 2}") for m in mcs}
                                    for j in range(KC // 2):
                                        for m in mcs:
                                            nc.tensor.matmul(
                                                pss[m][:],
                                                xq[:, 2 * j:2 * j + 2,
                                                   m * P:(m + 1) * P],
                                                wv_t[:, 2 * j:2 * j + 2, :],
                                                start=(j == 0),
                                                stop=(j == KC // 2 - 1),
                                                perf_mode=DR,
                                                skip_group_check=True)
                                    for m in mcs:
                                        nc.vector.tensor_scalar_mul(
                                            vv[:, m, ncol * 512:(ncol + 1) * 512],
                                            pss[m][:], rstd1_c[:, m:m + 1])
                            else:
                                for mc in range(MC):
                                    ps = ps_pool.tile([P, 512], F32, tag="vps",
                                                      name="vps")
                                    for j in range(KC // 2):
                                        nc.tensor.matmul(
                                            ps[:],
                                            xq[:, 2 * j:2 * j + 2,
                                               mc * P:(mc + 1) * P],
                                            wv_t[:, 2 * j:2 * j + 2, :],
                                            start=(j == 0),
                                            stop=(j == KC // 2 - 1),
                                            perf_mode=DR)
                                    nc.vector.tensor_scalar_mul(
                                        vv[:, mc, ncol * 512:(ncol + 1) * 512],
                                        ps[:], rstd1_c[:, mc:mc + 1])
                        hv = wv_pool.tile([P, DIM], BF16, tag="halo_v")
                        nc.sync.dma_start(out=hv[:], in_=halo_v[:])
                        nc.vector.tensor_add(vv[:, 0, :], vv[:, 0, :], hv[:])

                # ---- Phase 3: K = (Wk.T @ x) * m1 (dim x tok layout) ---------
                if "3" in phases:
                    with ExitStack() as ph:
                        ph.enter_context(nc.named_scope("ph3_k"))
                        w_pool = ph.enter_context(tc.tile_pool(name="wkp", bufs=3))
                        ps_pool = ph.enter_context(
                            tc.tile_pool(name="k_ps", bufs=2 if qkv_ilv else 4,
                                         space="PSUM"))
                        for oc in range(KC):
                            wk_t = w_pool.tile([P, KC, P], E4, tag="wk",
                                               name="wk")
                            wdma_start(out=wk_t[:], in_=wk_tl[oc])
                            if qkv_ilv:
                                pss = [ps_pool.tile([P, 512], F32,
                                                    tag=f"kps{i}",
                                                    name=f"kps{i}")
                                       for i in range(3)]
                                for j in range(KC // 2):
                                    for i, (c0, c1_) in enumerate(ext_cuts):
                                        nc.tensor.matmul(
                                            pss[i][:, :c1_ - c0],
                                            wk_t[:, 2 * j:2 * j + 2, :],
                                            xq[:, 2 * j:2 * j + 2, c0:c1_],
                                            start=(j == 0),
                                            stop=(j == KC // 2 - 1),
                                            perf_mode=DR,
                                            skip_group_check=True)
                                for i, (c0, c1_) in enumerate(ext_cuts):
                                    nc.vector.tensor_mul(kT[:, oc, c0:c1_],
                                                         pss[i][:, :c1_ - c0],
                                                         rstd1_b[:, c0:c1_])
                            else:
                                for (c0, c1_) in ext_cuts:
                                    ps = ps_pool.tile([P, 512], F32, tag="kps",
                                                      name="kps")
                                    for j in range(KC // 2):
                                        nc.tensor.matmul(
                                            ps[:, :c1_ - c0],
                                            wk_t[:, 2 * j:2 * j + 2, :],
                                            xq[:, 2 * j:2 * j + 2, c0:c1_],
                                            start=(j == 0),
                                            stop=(j == KC // 2 - 1),
                                            perf_mode=DR)
                                    nc.vector.tensor_mul(kT[:, oc, c0:c1_],
                                                         ps[:, :c1_ - c0],
                                                         rstd1_b[:, c0:c1_])
                        hk = w_pool.tile([P, H, W], BF16, tag="halo_k")
                        nc.sync.dma_start(out=hk[:], in_=halo_kT[:])
                        nc.vector.tensor_add(kT[:, :, 0:W], kT[:, :, 0:W], hk[:])

                # ---- Phase 4: per head: Q then band attention ----------------
                if "4" in phases:
                    ao_pool = mha.enter_context(tc.tile_pool(name="aop", bufs=1))
                    aoT = ao_pool.tile([P, KC, OWN], E4, tag="aoT")
                    with ExitStack() as ph:
                        ph.enter_context(nc.named_scope("ph4_attn"))
                        w_pool = ph.enter_context(tc.tile_pool(name="wqp", bufs=3))
                        qh_pool = ph.enter_context(tc.tile_pool(name="qhp", bufs=2))
                        sm_pool = ph.enter_context(tc.tile_pool(name="smp", bufs=3))
                        p_pool = ph.enter_context(tc.tile_pool(name="ppp", bufs=3))
                        q_ps = ph.enter_context(
                            tc.tile_pool(name="q_ps", bufs=1 if qkv_ilv else 2,
                                         space="PSUM"))
                        sc_ps = ph.enter_context(
                            tc.tile_pool(name="sc_ps", bufs=2, space="PSUM"))
                        tp_ps = ph.enter_context(
                            tc.tile_pool(name="tp_ps", bufs=2, space="PSUM"))
                        av_ps = ph.enter_context(
                            tc.tile_pool(name="av_ps", bufs=2, space="PSUM"))
                        for h in range(H):
                            wq_t = w_pool.tile([P, KC, P], E4, tag="wq",
                                               name="wq")
                            wdma_start(out=wq_t[:], in_=wq_tl[h])
                            qh = qh_pool.tile([P, OWN], BF16, tag="qh", name="qh")
                            if qkv_ilv:
                                qps = [q_ps.tile([P, 512], F32, tag=f"qps{ci}",
                                                 name=f"qps{ci}")
                                       for ci in range(2)]
                                for j in range(KC // 2):
                                    for ci, (c0, c1_) in enumerate(own_cuts):
                                        nc.tensor.matmul(
                                            qps[ci][:],
                                            wq_t[:, 2 * j:2 * j + 2, :],
                                            xq[:, 2 * j:2 * j + 2,
                                               W + c0:W + c1_],
                                            start=(j == 0),
                                            stop=(j == KC // 2 - 1),
                                            perf_mode=DR,
                                            skip_group_check=True)
                                for ci, (c0, c1_) in enumerate(own_cuts):
                                    nc.vector.tensor_mul(
                                        qh[:, c0:c1_], qps[ci][:],
                                        rstd1_b[:, W + c0:W + c1_])
                            else:
                                for (c0, c1_) in own_cuts:
                                    ps = q_ps.tile([P, 512], F32, tag="qps",
                                                   name="qps")
                                    for j in range(KC // 2):
                                        nc.tensor.matmul(
                                            ps[:],
                                            wq_t[:, 2 * j:2 * j + 2, :],
                                            xq[:, 2 * j:2 * j + 2,
                                               W + c0:W + c1_],
                                            start=(j == 0),
                                            stop=(j == KC // 2 - 1),
                                            perf_mode=DR)
                                    nc.vector.tensor_mul(
                                        qh[:, c0:c1_], ps[:],
                                        rstd1_b[:, W + c0:W + c1_])
                            for n in range(NBLK):
                                sc = sc_ps.tile([P, 2 * W], F32, tag="sc",
                                                name="sc")
                                nc.tensor.matmul(sc[:], qh[:, n * W:(n + 1) * W],
                                                 kT[:, h, n * W:n * W + 2 * W],
                                                 start=True, stop=True)
                                sm = sm_pool.tile([P, 2 * W], F32, tag="sm",
                                                  name="sm")
                                nc.vector.scalar_tensor_tensor(
                                    out=sm[:], in0=sc[:], scalar=SCALE, in1=mask[:],
                                    op0=mybir.AluOpType.mult,
                                    op1=mybir.AluOpType.add)
                                pe = p_pool.tile([P, 2 * W], BF16, tag="pe",
                                                 name="pe")
                                sumexp = sm_pool.tile([P, 1], F32, tag="sumexp",
                                                      name="sumexp")
                                nc.scalar.activation(
                                    out=pe[:], in_=sm[:],
                                    func=mybir.ActivationFunctionType.Exp,
                                    accum_out=sumexp[:])
                                recip = sm_pool.tile([P, 1], F32, tag="recip",
                                                     name="recip")
                                nc.vector.reciprocal(recip[:], sumexp[:])
                                pn = p_pool.tile([P, 2 * W], BF16, tag="pn",
                                                 name="pn")
                                nc.vector.tensor_scalar_mul(pn[:], pe[:], recip[:])
                                pT = p_pool.tile([P, 2, W], BF16, tag="pT",
                                                 name="pT")
                                for c in range(2):
                                    tp = tp_ps.tile([P, W], BF16, tag="tp",
                                                    name="tp")
                                    nc.tensor.transpose(
                                        tp[:], pn[:, c * W:(c + 1) * W], ident[:])
                                    nc.vector.tensor_copy(pT[:, c, :], tp[:])
                                av = av_ps.tile([P, W], F32, tag="av", name="av")
                                for c in range(2):
                                    nc.tensor.matmul(
                                        av[:], vv[:, n + c, h * HD:(h + 1) * HD],
                                        pT[:, c, :], start=(c == 0), stop=(c == 1))
                                nc.scalar.activation(
                                    out=aoT[:, h, n * W:(n + 1) * W], in_=av[:],
                                    func=mybir.ActivationFunctionType.Copy,
                                    scale=SX)

                # ---- Phase 5: O proj + residual -> x2sb (SBUF bf16), --------
                # ---- fused rmsnorm2 stats -----------------------------------
                if "5" in phases:
                    with ExitStack() as ph:
                        ph.enter_context(nc.named_scope("ph5_o"))
                        w_pool = ph.enter_context(tc.tile_pool(name="wop", bufs=3))
                        st_pool = ph.enter_context(tc.tile_pool(name="ost", bufs=3))
                        ps_pool = ph.enter_context(
                            tc.tile_pool(name="o_ps", bufs=3 if qkv_ilv else 4,
                                         space="PSUM"))
                        ss_ps = ph.enter_context(
                            tc.tile_pool(name="ss2_ps", bufs=1, space="PSUM"))
                        ss2 = [ss_ps.tile([1, 512], F32, tag=f"ss2_{i}",
                                          name=f"ss2_{i}") for i in range(2)]
                        for oc in range(KC):
                            wo_t = w_pool.tile([P, KC, P], E4, tag="wo", name="wo")
                            wdma_start(out=wo_t[:], in_=wo_tl[oc])
                            if qkv_ilv:
                                psb = [ps_pool.tile([P, 512], F32,
                                                    tag=f"ops{ci}",
                                                    name=f"ops{ci}")
                                       for ci in range(2)]
                                for j in range(KC // 2):
                                    for ci, (c0, c1_) in enumerate(own_cuts):
                                        nc.tensor.matmul(
                                            psb[ci][:],
                                            wo_t[:, 2 * j:2 * j + 2, :],
                                            aoT[:, 2 * j:2 * j + 2, c0:c1_],
                                            start=(j == 0),
                                            stop=(j == KC // 2 - 1),
                                            perf_mode=DR,
                                            skip_group_check=True)
                            for i, (c0, c1_) in enumerate(own_cuts):
                                if qkv_ilv:
                                    ps = psb[i]
                                else:
                                    ps = ps_pool.tile([P, 512], F32, tag="ops",
                                                      name="ops")
                                    for j in range(KC // 2):
                                        nc.tensor.matmul(
                                            ps[:],
                                            wo_t[:, 2 * j:2 * j + 2, :],
                                            aoT[:, 2 * j:2 * j + 2, c0:c1_],
                                            start=(j == 0),
                                            stop=(j == KC // 2 - 1),
                                            perf_mode=DR)
                                xres = st_pool.tile([P, 512], BF16, tag="xres",
                                                    name="xres")
                                nc.sync.dma_start(
                                    out=xres[:],
                                    in_=xT[oc * P:(oc + 1) * P, W + c0:W + c1_])
                                nc.vector.scalar_tensor_tensor(
                                    out=x2sb[:, oc, c0:c1_], in0=ps[:],
                                    scalar=MO, in1=xres[:],
                                    op0=mybir.AluOpType.mult,
                                    op1=mybir.AluOpType.add)
                                xsq = st_pool.tile([P, 512], BF16, tag="xsq2",
                                                   name="xsq2")
                                nc.vector.tensor_mul(xsq[:], x2sb[:, oc, c0:c1_],
                                                     x2sb[:, oc, c0:c1_])
                                nc.tensor.matmul(ss2[i][:], ones1[:], xsq[:],
                                                 start=(oc == 0),
                                                 stop=(oc == KC - 1),
                                                 skip_group_check=True)
                        rstd2 = st_pool.tile([1, OWN], F32, tag="rstd2")
                        for i, (c0, c1_) in enumerate(own_cuts):
                            nc.scalar.activation(
                                out=rstd2[:, c0:c1_], in_=ss2[i][:],
                                func=mybir.ActivationFunctionType.Sqrt,
                                bias=eps2_t[:], scale=c2)
                        nc.vector.reciprocal(rstd2[:], rstd2[:])
                        nc.gpsimd.partition_broadcast(rstd2_b[:], rstd2[:])

            # ---- Phase 6: SwiGLU FFN + residual ------------------------------
            if "6" in phases:
                # Single-pass bf16 FFN: weights stream once; hbuf for all
                # OWN tokens stays resident (128KB/partition); x2sb is
                # normalized in place (t2), residual recovered via irstd2_b.
                with ExitStack() as ph:
                    h_pool = ph.enter_context(tc.tile_pool(name="hbufp", bufs=1))
                    hbuf = h_pool.tile([P, KF, OWN], BF16, tag="hbuf")
                    wfo_pool = ph.enter_context(tc.tile_pool(name="wfop", bufs=1))

                    with ExitStack() as phfc:
                        phfc.enter_context(nc.named_scope("ph6a_fc"))
                        wf_pool = phfc.enter_context(
                            tc.tile_pool(name="wfp", bufs=2))
                        s_pool = phfc.enter_context(
                            tc.tile_pool(name="silp", bufs=3))
                        ps12_pool = phfc.enter_context(
                            tc.tile_pool(name="f_ps12", bufs=2, space="PSUM"))

                        # t2 = x2 * rstd2, in place (x2sb now holds t2);
                        # then rstd2_b inverted in place for the 6b residual
                        for kc in range(KC):
                            nc.vector.tensor_mul(x2sb[:, kc, :], x2sb[:, kc, :],
                                                 rstd2_b[:])
                        t2 = x2sb
                        nc.vector.reciprocal(rstd2_b[:], rstd2_b[:])

                        for oc in range(KF):
                            wfa_t = wf_pool.tile([P, KC, P], BF16, tag="wfa",
                                                 name="wfa")
                            nc.scalar.dma_start(out=wfa_t[:], in_=wfa_tl[oc])
                            wfc_t = wf_pool.tile([P, KC, P], BF16, tag="wfc",
                                                 name="wfc")
                            nc.sync.dma_start(out=wfc_t[:], in_=wfc_tl[oc])
                            if ffn_ilv == "4":
                                pss = {}
                                for ci, (c0, c1_) in enumerate(own_cuts):
                                    pss[ci] = [
                                        ps12_pool.tile([P, 512], F32,
                                                       tag=f"p{g}c{ci}",
                                                       name=f"p{g}c{ci}")
                                        for g in (1, 2)]
                                for kc in range(KC):
                                    for ci, (c0, c1_) in enumerate(own_cuts):
                                        nc.tensor.matmul(
                                            pss[ci][0][:], wfa_t[:, kc, :],
                                            t2[:, kc, c0:c1_],
                                            start=(kc == 0),
                                            stop=(kc == KC - 1),
                                            skip_group_check=True)
                                        nc.tensor.matmul(
                                            pss[ci][1][:], wfc_t[:, kc, :],
                                            t2[:, kc, c0:c1_],
                                            start=(kc == 0),
                                            stop=(kc == KC - 1),
                                            skip_group_check=True)
                                for ci, (c0, c1_) in enumerate(own_cuts):
                                    sil = s_pool.tile([P, 512], BF16, tag="sil",
                                                      name="sil")
                                    nc.scalar.activation(
                                        out=sil[:], in_=pss[ci][0][:],
                                        func=mybir.ActivationFunctionType.Silu)
                                    nc.vector.tensor_mul(hbuf[:, oc, c0:c1_],
                                                         pss[ci][1][:], sil[:])
                                continue
                            for (c0, c1_) in own_cuts:
                                ps1 = ps12_pool.tile([P, 512], F32, tag="ps1",
                                                     name="ps1")
                                ps2 = ps12_pool.tile([P, 512], F32, tag="ps2",
                                                     name="ps2")
                                if ffn_ilv:
                                    for kc in range(KC):
                                        nc.tensor.matmul(
                                            ps1[:], wfa_t[:, kc, :],
                                            t2[:, kc, c0:c1_],
                                            start=(kc == 0),
                                            stop=(kc == KC - 1),
                                            skip_group_check=True)
                                        nc.tensor.matmul(
                                            ps2[:], wfc_t[:, kc, :],
                                            t2[:, kc, c0:c1_],
                                            start=(kc == 0),
                                            stop=(kc == KC - 1),
                                            skip_group_check=True)
                                else:
                                    for kc in range(KC):
                                        nc.tensor.matmul(
                                            ps1[:], wfa_t[:, kc, :],
                                            t2[:, kc, c0:c1_],
                                            start=(kc == 0),
                                            stop=(kc == KC - 1))
                                    for kc in range(KC):
                                        nc.tensor.matmul(
                                            ps2[:], wfc_t[:, kc, :],
                                            t2[:, kc, c0:c1_],
                                            start=(kc == 0),
                                            stop=(kc == KC - 1))
                                sil = s_pool.tile([P, 512], BF16, tag="sil",
                                                  name="sil")
                                nc.scalar.activation(
                                    out=sil[:], in_=ps1[:],
                                    func=mybir.ActivationFunctionType.Silu)
                                nc.vector.tensor_mul(hbuf[:, oc, c0:c1_],
                                                     ps2[:], sil[:])

                    with ExitStack() as phfo:
                        phfo.enter_context(nc.named_scope("ph6b_out"))
                        y_pool = phfo.enter_context(
                            tc.tile_pool(name="yp", bufs=3))
                        ps3_pool = phfo.enter_context(
                            tc.tile_pool(name="f_ps3", bufs=3, space="PSUM"))
                        for oc in range(KC):
                            wfo_t = [wfo_pool.tile([P, KF // 4, P], BF16,
                                                   tag=f"wfo{q}",
                                                   name=f"wfo{q}")
                                     for q in range(4)]
                            for q in range(4):
                                weng = nc.scalar if q % 2 == 0 else nc.sync
                                weng.dma_start(out=wfo_t[q][:],
                                               in_=wfo_tl[oc, q])
                            if ffn_ilv:
                                pso = [ps3_pool.tile([P, 512], F32,
                                                     tag=f"ps3c{ci}",
                                                     name=f"ps3c{ci}")
                                       for ci in range(2)]
                                for kc in range(KF):
                                    for ci, (c0, c1_) in enumerate(own_cuts):
                                        nc.tensor.matmul(
                                            pso[ci][:],
                                            wfo_t[kc // (KF // 4)]
                                            [:, kc % (KF // 4), :],
                                            hbuf[:, kc, c0:c1_],
                                            start=(kc == 0),
                                            stop=(kc == KF - 1),
                                            skip_group_check=True)
                                for ci, (c0, c1_) in enumerate(own_cuts):
                                    x2r = y_pool.tile([P, 512], BF16, tag="x2r",
                                                      name="x2r")
                                    nc.vector.tensor_mul(x2r[:],
                                                         t2[:, oc, c0:c1_],
                                                         rstd2_b[:, c0:c1_])
                                    yt = y_pool.tile([P, 512], F32, tag="yt",
                                                     name="yt")
                                    nc.vector.tensor_add(yt[:], pso[ci][:],
                                                         x2r[:])
                                    nc.gpsimd.dma_start(
                                        out=yT[oc * P:(oc + 1) * P, c0:c1_],
                                        in_=yt[:])
                                continue
                            for (c0, c1_) in own_cuts:
                                ps = ps3_pool.tile([P, 512], F32, tag="ps3",
                                                   name="ps3")
                                for kc in range(KF):
                                    nc.tensor.matmul(
                                        ps[:],
                                        wfo_t[kc // (KF // 4)][:, kc % (KF // 4), :],
                                        hbuf[:, kc, c0:c1_],
                                        start=(kc == 0), stop=(kc == KF - 1))
                                x2r = y_pool.tile([P, 512], BF16, tag="x2r",
                                                  name="x2r")
                                nc.vector.tensor_mul(x2r[:], t2[:, oc, c0:c1_],
                                                     rstd2_b[:, c0:c1_])
                                yt = y_pool.tile([P, 512], F32, tag="yt",
                                                 name="yt")
                                nc.vector.tensor_add(yt[:], ps[:], x2r[:])
                                nc.gpsimd.dma_start(
                                    out=yT[oc * P:(oc + 1) * P, c0:c1_],
                                    in_=yt[:])

    nc.compile()
    return nc


def _tile_w(wt, nkc, noc, ocw):
    """(din, dout) -> (dout//ocw, 128, din//128, ocw) so each [oc] is contiguous."""
    return np.ascontiguousarray(
        wt.reshape(nkc, P, noc, ocw).transpose(2, 1, 0, 3))


def _q8(wt):
    return np.clip(wt * SW, -240.0, 240.0).astype(F8)


def _prep_inputs(x, wq, wk, wv, wo, last_k_init, last_v_init,
                 w_fc, w_fc_act, w_fc_out, g_mha, g_ffn):
    wq_t = _q8((wq * g_mha[None, :]).T)
    wk_t = _q8((wk * g_mha[None, :]).T)
    wv_t = _q8((wv * g_mha[None, :]).T)
    wo_t = _q8(wo.T)
    wfa_t = ((w_fc_act * g_ffn[None, :]).T).astype(BF)
    wfc_t = ((w_fc * g_ffn[None, :]).T).astype(BF)
    wfo_t = w_fc_out.T.astype(BF)

    wfo_tl = _tile_w(wfo_t, KF, KC, P)          # [KC, P, KF, P]
    wfo_tl = np.ascontiguousarray(
        wfo_tl.reshape(KC, P, 4, KF // 4, P).transpose(0, 2, 1, 3, 4))

    shared = {
        "wq_tl": _tile_w(wq_t, KC, KC, P),
        "wk_tl": _tile_w(wk_t, KC, KC, P),
        "wv_tl": _tile_w(wv_t, KC, 4, 512),
        "wo_tl": _tile_w(wo_t, KC, KC, P),
        "wfa_tl": _tile_w(wfa_t, KC, KF, P),
        "wfc_tl": _tile_w(wfc_t, KC, KF, P),
        "wfo_tl": wfo_tl,
    }

    # halo k/v for first-chunk cores, from last_k/v_init
    hk = np.zeros((W, H, HD), np.float32)
    hk[1:W] = last_k_init
    halo_kT0 = np.ascontiguousarray(hk.transpose(2, 1, 0)).astype(BF)  # (hd,h,j)
    hv = np.zeros((W, DIM), np.float32)
    hv[1:W] = last_v_init.reshape(W - 1, DIM)
    halo_v0 = hv.astype(BF)
    halo_kTz = np.zeros_like(halo_kT0)
    halo_vz = np.zeros_like(halo_v0)

    in_maps = []
    for c in range(NCORES):
        b, s = divmod(c * OWN, L)
        xe = np.zeros((EXT, DIM), np.float32)
        xe[W:] = x[b, s:s + OWN]
        if s > 0:
            xe[:W] = x[b, s - W:s]
        m = dict(shared)
        m["xT"] = np.ascontiguousarray(xe.T).astype(BF)
        m["halo_kT"] = halo_kT0 if s == 0 else halo_kTz
        m["halo_v"] = halo_v0 if s == 0 else halo_vz
        in_maps.append(m)
    return in_maps


def _run(inputs, trace=False, **build_kwargs):
    key = tuple(sorted(build_kwargs.items()))
    if ("nc", key) not in _CACHE:
        _CACHE[("nc", key)] = _build(**build_kwargs)
    nc = _CACHE[("nc", key)]
    in_maps = _prep_inputs(**{k: np.asarray(v) for k, v in inputs.items()})
    res = run_bass_kernel_spmd(nc, in_maps, core_ids=list(range(NCORES)),
                               trace=trace)
    y = np.empty((B, L, DIM), np.float32)
    for c in range(NCORES):
        b, s = divmod(c * OWN, L)
        y[b, s:s + OWN] = res.results[c]["yT"].T
    return y, res


def kernel(**inputs):
    y, _ = _run(inputs, trace=False)
    return y
